# revision 54
# baseline (speedup 1.0000x reference)
"""BatchMultiHeadGraphAttention TRN2 kernel (Z9 design).

Reference computation (per batch b, head h):
    h_prime = h[b] @ w[h]                          # [n, f]
    t = tanh(h_prime)
    src = t @ a_src[h];  dst = t @ a_dst[h]        # [n]
    s[i, j] = leaky_relu(src[i] + dst[j], 0.2)
    attn = softmax(where(adj[b] | eye, s, -inf), axis=-1)
    out[b, h] = attn @ h_prime + bias

Sharding: 8 cores, one (b, h) slab per core (bs=4 x H=2).

Key algebra (per-column-i scales cancel in softmax; exp is monotone so
exp(lrelu(s)) = max(e^s, e^{0.2s})):

    P[j,i] = adj' * max(Esrc_i*Edst_j, E2src_i*E2dst_j)
           = Esrc_i * Edst_j * adj'[j,i] * max(1, q_i*qd_j)
      q_i = exp(-0.8*src_i), qd_j = exp(-0.8*dst_j); Esrc_i cancels.

PV matmul operands (both fp8e4m3 -> DoubleRow perf mode, 0.5 cyc/row,
two j-tiles per instruction):
    lhsT  Pt[j,i] = f8(adj' * max(1, q_i*qd_j))   # non-leaky entries are
                                                  # exactly 1.0 on the grid
    rhs   G1[j,f] = f8(Edst_j * h_prime[j,f])  (+ Z column = Edst_j)
    out[i] = (Pt^T @ G1)[i, :768] / (Pt^T @ G1)[i, Z]
A second accumulation chain Pt^T @ f8(G1 - f8(G1)) over the first
KRES8 (default 6) of 8 tile-pairs cancels most of the rhs fp8 noise:
rel err 1.38e-2 (full res 1.03e-2, none 1.9e-2) vs the 2e-2 gate.

All n-indices on chip live in the sigma order induced by the u16-pair
xbar transpose of adj (sigma(128*tau+p) = 256*(tau//2) + 2p + tau%2),
applied to j and i alike so the diagonal stays the diagonal; h is
DMA-loaded in sigma row order and the output DMA un-permutes.

Scheduling notes: engine queues are in-order and the tile scheduler
coalesces cross-engine waits into coarse engine-counter thresholds, so
ordering matters: h/w/adj loads are interleaved with just-in-time PE
transposes, the per-tau phase-2 chain keeps only dots on DVE and the
G1 residual subtract on Pool, q broadcasts are per-group tiles (no
whole-tensor deps), and masks are emitted post-loop group-by-group
(alternating DVE/Pool) staggered with the PV chain emission.
"""

import os
import numpy as np

BS, N, H, F_IN, F_OUT = 4, 2048, 2, 768, 768
NCORES = 8

_CACHE = {}


def _build(has_bias: bool):
    import concourse.bass as bass
    import concourse.mybir as mybir
    import concourse.tile as tile
    from concourse import bacc
    from concourse.masks import make_identity

    dt = mybir.dt
    AF = mybir.ActivationFunctionType
    OP = mybir.AluOpType
    PM = mybir.MatmulPerfMode

    KRES = int(os.environ.get("KRES", "1"))       # G1 residual chain
    KRES8 = int(os.environ.get("KRES8", "6"))     # res s-pairs (of 8)
    KNG = int(os.environ.get("KNG", "4"))         # i groups
    KPOOLG = int(os.environ.get("KPOOLG", "2"))   # mask groups on Pool

    NT = N // 128            # 16 n-tiles
    KT = F_IN // 128         # 6 k-tiles
    NG = KNG
    BB = (N // 256) // NG    # 256-blocks per group
    GW = N // NG             # group width
    CHG = GW // 128          # i-chunks per group
    GZ = F_OUT + 4           # G1 row: 768 vals + Z col + pad
    ZC = F_OUT               # Z column index

    nc = bacc.Bacc("TRN2", target_bir_lowering=False, debug=False,
                   num_devices=NCORES)

    d_h = nc.dram_tensor("h", [N, F_IN], dt.float32, kind="ExternalInput")
    d_adj = nc.dram_tensor("adj", [N, N], dt.uint8, kind="ExternalInput")
    d_w = nc.dram_tensor("w", [F_IN, F_OUT], dt.float32, kind="ExternalInput")
    d_asrc = nc.dram_tensor("a_src", [F_OUT], dt.float32, kind="ExternalInput")
    d_adst = nc.dram_tensor("a_dst", [F_OUT], dt.float32, kind="ExternalInput")
    if has_bias:
        d_bias = nc.dram_tensor("bias", [F_OUT], dt.float32,
                                kind="ExternalInput")
    d_out = nc.dram_tensor("out", [N, F_OUT], dt.float32,
                           kind="ExternalOutput")

    def sig_off(tau):
        return 256 * (tau // 2) + (tau % 2)

    # Mask (g, tau) pairs: Pool takes the first KPOOLG groups entirely
    # (emitted at their gates, running under phase 2); DVE takes the rest
    # after the phase-2 loop so the dot chain isn't stalled.  The last
    # Pool group is emitted post-loop too, overlapping PV of the earlier
    # groups.
    def gate(g, tau):
        return max(4 * g + 3, tau)

    with tile.TileContext(nc) as tc:
        with tc.tile_pool(name="const", bufs=1) as cpool, \
             tc.tile_pool(name="persist", bufs=1) as pp:
            identb = cpool.tile([128, 128], dt.bfloat16, tag="identb")
            make_identity(nc, identb[:])
            ident32 = cpool.tile([128, 128], dt.float32, tag="ident32")
            make_identity(nc, ident32[:])
            eye_u8 = cpool.tile([128, 128], dt.uint8, tag="eye_u8")
            nc.vector.tensor_copy(eye_u8[:], identb[:])

            if has_bias:
                bias_row = cpool.tile([1, F_OUT], dt.float32, tag="bias_row")
                nc.sync.dma_start(bias_row[:],
                                  d_bias.ap().rearrange("(o f) -> o f", o=1))
                bias_bc = pp.tile([128, F_OUT], dt.float32, tag="bias_bc")
                nc.gpsimd.partition_broadcast(bias_bc[:], bias_row[:])

            # hT split per 4-tile n-group so phase 2 can start early
            hT = [[pp.tile([128, 512], dt.bfloat16, tag=f"hT{g2}_{k}",
                           name=f"hT{g2}_{k}")
                   for k in range(KT)] for g2 in range(NT // 4)]
            w_bf = pp.tile([128, KT, F_OUT], dt.bfloat16, tag="w_bf")
            asrc_bc = pp.tile([128, F_OUT], dt.bfloat16, tag="asrc_bc")
            adst_bc = pp.tile([128, F_OUT], dt.bfloat16, tag="adst_bc")

            stag = [pp.tile([128, N], dt.uint16, tag=f"stag{t}",
                            name=f"stag{t}")
                    for t in range(NT // 2)]
            adj16 = d_adj.ap().bitcast(dt.uint16)

            g1hi = [pp.tile([128, 2, GZ], dt.float8e4, tag=f"g1hi{s}",
                            name=f"g1hi{s}")
                    for s in range(NT // 2)]
            if KRES:
                g1res = [pp.tile([128, 2, GZ], dt.float8e4, tag=f"g1res{s}",
                                 name=f"g1res{s}")
                         for s in range(KRES8)]
            # Pt tiles: all NG groups live at once (mask building overlaps
            # phase 2 and PV consumes group-by-group)
            P2 = [[pp.tile([128, 2, GW], dt.float8e4, tag=f"P2_{g}_{s}",
                           name=f"P2_{g}_{s}")
                   for s in range(NT // 2)] for g in range(NG)]

            src_col = pp.tile([128, NT], dt.float32, tag="src_col")
            dst_col = pp.tile([128, NT], dt.float32, tag="dst_col")
            edst_col = pp.tile([128, NT], dt.float32, tag="edst_col")
            q_bc = [pp.tile([128, GW], dt.bfloat16, tag=f"q_bc{g}",
                            name=f"q_bc{g}")
                    for g in range(NG)]
            qd_col = [pp.tile([128, 4], dt.float32, tag=f"qd_col{q}",
                              name=f"qd_col{q}")
                      for q in range(NT // 4)]
            q_flat = [pp.tile([1, GW], dt.bfloat16, tag=f"q_flat{g}",
                              name=f"q_flat{g}")
                      for g in range(NG)]
            rz_col = pp.tile([128, NT], dt.float32, tag="rz_col")

            for s in range(NT // 2):
                nc.gpsimd.memset(g1hi[s][:, :, ZC + 1:], 0.0)
                if KRES and s < KRES8:
                    nc.gpsimd.memset(g1res[s][:, :, ZC + 1:], 0.0)

            def u8view(t):
                return stag[t][:].bitcast(dt.uint8).rearrange(
                    "p (b k hh o) -> p b hh k o", b=NT // 2, k=128, hh=2, o=2)

            def emit_diag(tau, dpool):
                # diag of tile tau lives in group g = tau // CHG at m-offset
                # 128*tau; recompute the 128-wide D1 diag block (cheap) and
                # write it where eye is set (the adj|eye diagonal fix)
                g = tau // CHG
                t, o = tau // 2, tau % 2
                dd = dpool.tile([128, 128], dt.float8e4, tag="dd")
                qslice = q_bc[g][:, 128 * tau - GW * g:
                                128 * (tau + 1) - GW * g]
                nc.vector.tensor_scalar(
                    dd[:], qslice,
                    qd_col[tau // 4][:, tau % 4:tau % 4 + 1], 1.0,
                    op0=OP.mult, op1=OP.max)
                nc.vector.copy_predicated(
                    P2[g][t][:, o, 128 * tau - GW * g:
                             128 * (tau + 1) - GW * g], eye_u8[:], dd[:])

            def emit_d1(g, tau, d1pool):
                d1 = d1pool.tile([128, GW], dt.bfloat16, tag="d1")
                nc.vector.tensor_scalar(
                    d1[:], q_bc[g][:],
                    qd_col[tau // 4][:, tau % 4:tau % 4 + 1], 1.0,
                    op0=OP.mult, op1=OP.max)
                return d1

            def emit_mask(g, tau, d1, on_pool):
                t, o = tau // 2, tau % 2
                u8v = u8view(t)
                d13 = d1[:].rearrange("p (b hh k) -> p b hh k",
                                      b=BB, hh=2, k=128)
                p23 = P2[g][t][:, o, :].rearrange(
                    "p (b hh k) -> p b hh k", b=BB, hh=2, k=128)
                for hh in range(2):
                    in0 = u8v[:, BB * g:BB * (g + 1), hh, :, o]
                    eng = nc.gpsimd if on_pool else nc.vector
                    eng.tensor_tensor(p23[:, :, hh, :], in0, d13[:, :, hh, :],
                                      op=OP.mult)

            with tc.tile_pool(name="ph1", bufs=1) as hpool, \
                 tc.tile_pool(name="ph1ps", bufs=2, space="PSUM") as psum1, \
                 tc.tile_pool(name="ph2", bufs=3) as tpool, \
                 tc.tile_pool(name="ph2s", bufs=3) as spool, \
                 tc.tile_pool(name="ph2ps", bufs=3, space="PSUM") as psum2:
                # ---- loads (h/w/adj on HWDGE; cast-DMAs occupy Pool, so
                # only the tiny a-vectors use them) ----
                arow = hpool.tile([1, 2 * F_OUT], dt.bfloat16, tag="arow")
                nc.gpsimd.dma_start(
                    arow[:, 0:F_OUT],
                    d_asrc.ap().rearrange("(o f) -> o f", o=1))
                nc.gpsimd.dma_start(
                    arow[:, F_OUT:],
                    d_adst.ap().rearrange("(o f) -> o f", o=1))
                nc.gpsimd.partition_broadcast(asrc_bc[:], arow[:, 0:F_OUT])
                nc.gpsimd.partition_broadcast(adst_bc[:], arow[:, F_OUT:])

                # h in sigma row order: rows 2p,2p+1 are HBM-adjacent so
                # (o f) merges into one contiguous 1536-elem run -> one DMA
                # per 4-tile group.  Order: h group 0 first (transposes),
                # then w (whole, one DMA + one DVE copy at the head of the
                # DVE queue), then the rest interleaved with adj xbar.
                dh3 = d_h.ap().rearrange("(b p o) f -> p b (o f)",
                                         b=NT // 2, p=128, o=2)
                hstage = []

                def load_h(g2):
                    hs = hpool.tile([128, 4, F_IN], dt.float32,
                                    tag=f"hs{g2 % 2}", name=f"hs{g2}")
                    nc.sync.dma_start(
                        hs[:].rearrange("p (b o) f -> p b (o f)", b=2, o=2),
                        dh3[:, 2 * g2:2 * g2 + 2, :])
                    hstage.append(hs)

                load_h(0)
                wtmp = hpool.tile([128, KT * F_OUT], dt.float32, tag="wtmp")
                nc.sync.dma_start(
                    wtmp[:].rearrange("p (k f) -> p k f", k=KT),
                    d_w.ap().rearrange("(k p) f -> p k f", k=KT, p=128))
                wt3 = wtmp[:].rearrange("p (k f) -> p k f", k=KT)
                nc.vector.tensor_copy(w_bf[:, :, 0:512], wt3[:, :, 0:512])
                nc.vector.tensor_copy(w_bf[:, :, 512:F_OUT],
                                      wt3[:, :, 512:F_OUT])
                for g2 in range(1, NT // 4):
                    load_h(g2)
                    for t in range(2 * (g2 - 1), 2 * (g2 - 1) + 2):
                        nc.sync.dma_start(stag[t][:],
                                          adj16[:, 128 * t:128 * (t + 1)],
                                          transpose=True)
                for t in range(NT // 2 - 2, NT // 2):
                    nc.sync.dma_start(stag[t][:],
                                      adj16[:, 128 * t:128 * (t + 1)],
                                      transpose=True)

                # ---- phase 2 tau loop with gated mask interleaving ----
                # (hT transposes/evicts are emitted just-in-time per 4-tile
                # group so the ACT queue isn't blocked on late h loads)
                emitted = set()
                diag_todo = []

                for tau in range(NT):
                    s, o = tau // 2, tau % 2
                    if tau % 4 == 0:
                        g2 = tau // 4
                        hsr = hstage[g2][:]
                        for k in range(KT):
                            ps = psum1.tile([128, 512], dt.float32,
                                            tag="tps")
                            for nn in range(4):
                                nc.tensor.transpose(
                                    ps[:, 128 * nn:128 * (nn + 1)],
                                    hsr[:, nn, 128 * k:128 * (k + 1)],
                                    ident32[:])
                            if k % 3 == 2:
                                nc.vector.tensor_copy(hT[g2][k][:], ps[:])
                            else:
                                nc.scalar.copy(hT[g2][k][:], ps[:])
                    ps = psum2.tile([128, F_OUT], dt.float32, tag="hpps")
                    for k in range(KT):
                        lhsT = hT[tau // 4][k][:, 128 * (tau % 4):
                                               128 * (tau % 4 + 1)]
                        nc.tensor.matmul(ps[:, 0:512], lhsT,
                                         w_bf[:, k, 0:512],
                                         start=(k == 0), stop=(k == KT - 1))
                        nc.tensor.matmul(ps[:, 512:F_OUT], lhsT,
                                         w_bf[:, k, 512:F_OUT],
                                         start=(k == 0), stop=(k == KT - 1))
                    tnh = tpool.tile([128, F_OUT], dt.bfloat16, tag="tnh")
                    nc.scalar.activation(tnh[:], ps[:], AF.Tanh)
                    scr = tpool.tile([128, F_OUT], dt.bfloat16, tag="scr")
                    nc.vector.scalar_tensor_tensor(
                        scr[:], tnh[:], 1.0, asrc_bc[:],
                        op0=OP.mult, op1=OP.mult,
                        accum_out=src_col[:, tau:tau + 1])
                    scr2 = tpool.tile([128, F_OUT], dt.bfloat16, tag="scr2")
                    nc.vector.scalar_tensor_tensor(
                        scr2[:], tnh[:], 1.0, adst_bc[:],
                        op0=OP.mult, op1=OP.mult,
                        accum_out=dst_col[:, tau:tau + 1])
                    nc.scalar.activation(edst_col[:, tau:tau + 1],
                                         dst_col[:, tau:tau + 1], AF.Exp)
                    nc.scalar.activation(
                        qd_col[tau // 4][:, tau % 4:tau % 4 + 1],
                        dst_col[:, tau:tau + 1], AF.Exp, scale=-0.8)
                    if has_bias:
                        nc.gpsimd.tensor_tensor(ps[:], ps[:], bias_bc[:],
                                                op=OP.add)
                    esl = edst_col[:, tau:tau + 1]
                    g1bf = tpool.tile([128, ZC + 1], dt.bfloat16, tag="g1bf")
                    nc.scalar.activation(g1bf[:, 0:F_OUT], ps[:],
                                         AF.Copy, scale=esl)
                    nc.scalar.copy(g1bf[:, ZC:ZC + 1], esl)
                    # f8 quantize on DVE (2x), residual on Pool
                    nc.vector.tensor_copy(g1hi[s][:, o, 0:ZC + 1], g1bf[:])
                    if KRES and s < KRES8:
                        nc.gpsimd.tensor_tensor(
                            g1res[s][:, o, 0:ZC + 1], g1bf[:],
                            g1hi[s][:, o, 0:ZC + 1], op=OP.subtract)
                    # q row slice for group g=tau//4 once its src dots exist
                    if tau % 4 == 3:
                        g = tau // 4
                        qb = spool.tile([128, 4], dt.bfloat16, tag="qb")
                        nc.scalar.activation(qb[:], src_col[:, 4 * g:4 * g + 4],
                                             AF.Exp, scale=-0.8)
                        psqt = psum1.tile([128, 512], dt.float32,
                                          tag="tps", name=f"psq{g}")
                        psq = psqt[0:4, 0:64].bitcast(dt.bfloat16)
                        nc.tensor.transpose(psq, qb[:], identb[:])
                        qsT = spool.tile([4, 128], dt.bfloat16, tag="qsT")
                        nc.scalar.copy(qsT[:], psq)
                        nc.sync.dma_start(
                            q_flat[g][:].rearrange("o (t p) -> o t p", t=4),
                            qsT[:])
                        nc.gpsimd.partition_broadcast(q_bc[g][:],
                                                      q_flat[g][:])

                # Post-loop: masks per group (alternating DVE/Pool),
                # staggered with PV emission so early PV chains don't sit
                # behind the last group's q-broadcast in the coarse
                # engine-counter waits.
                KPOOLM = int(os.environ.get("KPOOLM", "5"))

                def emit_group_masks(g, mpool):
                    for tp in range(NT):
                        on_pool = (tp % 16) < KPOOLM
                        emit_mask(g, tp, emit_d1(g, tp, mpool), on_pool)
                        emitted.add((g, tp))
                        if tp // CHG == g:
                            emit_diag(tp, mpool)

            # ---- PV: DoubleRow chains per (g, i-chunk) ----
            with tc.tile_pool(name="pgo", bufs=4) as opool, \
                 tc.tile_pool(name="pgm", bufs=3) as mpool, \
                 tc.tile_pool(name="pgps", bufs=8, space="PSUM") as psum5:
                FH = ((512, GZ - 2), (0, 512))  # Z-half first

                def emit_pv(g):
                    for c in range(CHG):
                        ci = CHG * g + c
                        for fh, (f0, f1) in enumerate(FH):
                            nf = f1 - f0
                            pso = psum5.tile([128, 512], dt.float32,
                                             tag="pso")
                            nchain = NT // 2 + (KRES8 if KRES else 0)
                            idx = 0
                            for s in range(NT // 2):
                                nc.tensor.matmul(
                                    pso[:, 0:nf],
                                    P2[g][s][:, :, 128 * c:128 * (c + 1)],
                                    g1hi[s][:, :, f0:f1],
                                    start=(idx == 0),
                                    stop=(idx == nchain - 1),
                                    perf_mode=PM.DoubleRow)
                                idx += 1
                            if KRES:
                                for s in range(KRES8):
                                    nc.tensor.matmul(
                                        pso[:, 0:nf],
                                        P2[g][s][:, :, 128 * c:128 * (c + 1)],
                                        g1res[s][:, :, f0:f1],
                                        start=False,
                                        stop=(idx == nchain - 1),
                                        perf_mode=PM.DoubleRow)
                                    idx += 1
                            if fh == 0:
                                nc.vector.reciprocal(
                                    rz_col[:, ci:ci + 1],
                                    pso[:, ZC - 512:ZC - 511])
                            ow = 256 if fh == 0 else 512
                            ob = opool.tile([128, ow], dt.float32,
                                            tag=f"ob{fh}")
                            nc.scalar.activation(
                                ob[:], pso[:, 0:ow], AF.Copy,
                                scale=rz_col[:, ci:ci + 1])
                            base = sig_off(ci)
                            orows = d_out[base:base + 255:2, f0:f0 + ow]
                            nc.sync.dma_start(orows, ob[:])

                KDRIVER = os.environ.get("KDRIVER", "MMPMPMPP")
                mq, pq = [0, 1, 2, 3], [0, 1, 2, 3]
                for ch in KDRIVER:
                    if ch == 'M' and mq:
                        emit_group_masks(mq.pop(0), mpool)
                    elif ch == 'P' and pq:
                        emit_pv(pq.pop(0))
                for g in mq:
                    emit_group_masks(g, mpool)
                for g in pq:
                    emit_pv(g)

    nc.compile()
    return nc


def _get_program(has_bias: bool):
    key = ("prog", has_bias)
    if key not in _CACHE:
        _CACHE[key] = _build(has_bias)
    return _CACHE[key]


def kernel(h, adj, w, a_src, a_dst, bias):
    from concourse.bass_utils import run_bass_kernel_spmd

    h = np.ascontiguousarray(np.asarray(h, dtype=np.float32))
    adj_u8 = np.ascontiguousarray(np.asarray(adj).astype(np.uint8))
    w = np.ascontiguousarray(np.asarray(w, dtype=np.float32))
    a_src = np.asarray(a_src, dtype=np.float32).reshape(H, F_OUT)
    a_dst = np.asarray(a_dst, dtype=np.float32).reshape(H, F_OUT)
    bias = np.asarray(bias, dtype=np.float32).reshape(F_OUT)
    has_bias = bool(np.any(bias))

    nc = _get_program(has_bias)

    in_maps = []
    for core in range(NCORES):
        b, hd = core // H, core % H
        m = {
            "h": h[b],
            "adj": adj_u8[b],
            "w": w[hd],
            "a_src": a_src[hd],
            "a_dst": a_dst[hd],
        }
        if has_bias:
            m["bias"] = bias
        in_maps.append(m)

    res = run_bass_kernel_spmd(nc, in_maps, list(range(NCORES)))
    out = np.empty((BS, H, N, F_OUT), dtype=np.float32)
    for core in range(NCORES):
        b, hd = core // H, core % H
        out[b, hd] = res.results[core]["out"]
    return out


# revision 55
# speedup vs baseline: 1.0092x; 1.0092x over previous
"""BatchMultiHeadGraphAttention TRN2 kernel (Z9 design).

Reference computation (per batch b, head h):
    h_prime = h[b] @ w[h]                          # [n, f]
    t = tanh(h_prime)
    src = t @ a_src[h];  dst = t @ a_dst[h]        # [n]
    s[i, j] = leaky_relu(src[i] + dst[j], 0.2)
    attn = softmax(where(adj[b] | eye, s, -inf), axis=-1)
    out[b, h] = attn @ h_prime + bias

Sharding: 8 cores, one (b, h) slab per core (bs=4 x H=2).

Key algebra (per-column-i scales cancel in softmax; exp is monotone so
exp(lrelu(s)) = max(e^s, e^{0.2s})):

    P[j,i] = adj' * max(Esrc_i*Edst_j, E2src_i*E2dst_j)
           = Esrc_i * Edst_j * adj'[j,i] * max(1, q_i*qd_j)
      q_i = exp(-0.8*src_i), qd_j = exp(-0.8*dst_j); Esrc_i cancels.

PV matmul operands (both fp8e4m3 -> DoubleRow perf mode, 0.5 cyc/row,
two j-tiles per instruction):
    lhsT  Pt[j,i] = f8(adj' * max(1, q_i*qd_j))   # non-leaky entries are
                                                  # exactly 1.0 on the grid
    rhs   G1[j,f] = f8(Edst_j * h_prime[j,f])  (+ Z column = Edst_j)
    out[i] = (Pt^T @ G1)[i, :768] / (Pt^T @ G1)[i, Z]
A second accumulation chain Pt^T @ f8(G1 - f8(G1)) over the first
KRES8 (default 6) of 8 tile-pairs cancels most of the rhs fp8 noise:
rel err 1.38e-2 (full res 1.03e-2, none 1.9e-2) vs the 2e-2 gate.

All n-indices on chip live in the sigma order induced by the u16-pair
xbar transpose of adj (sigma(128*tau+p) = 256*(tau//2) + 2p + tau%2),
applied to j and i alike so the diagonal stays the diagonal; h is
DMA-loaded in sigma row order and the output DMA un-permutes.

Scheduling notes: engine queues are in-order and the tile scheduler
coalesces cross-engine waits into coarse engine-counter thresholds, so
ordering matters: h/w/adj loads are interleaved with just-in-time PE
transposes, the per-tau phase-2 chain keeps only dots on DVE and the
G1 residual subtract on Pool, q broadcasts are per-group tiles (no
whole-tensor deps), and masks are emitted post-loop group-by-group
(alternating DVE/Pool) staggered with the PV chain emission.
"""

import os
import numpy as np

BS, N, H, F_IN, F_OUT = 4, 2048, 2, 768, 768
NCORES = 8

_CACHE = {}


def _build(has_bias: bool):
    import concourse.bass as bass
    import concourse.mybir as mybir
    import concourse.tile as tile
    from concourse import bacc
    from concourse.masks import make_identity

    dt = mybir.dt
    AF = mybir.ActivationFunctionType
    OP = mybir.AluOpType
    PM = mybir.MatmulPerfMode

    KRES = int(os.environ.get("KRES", "1"))       # G1 residual chain
    KRES8 = int(os.environ.get("KRES8", "6"))     # res s-pairs (of 8)
    KNG = int(os.environ.get("KNG", "4"))         # i groups
    KPOOLG = int(os.environ.get("KPOOLG", "2"))   # mask groups on Pool

    NT = N // 128            # 16 n-tiles
    KT = F_IN // 128         # 6 k-tiles
    NG = KNG
    BB = (N // 256) // NG    # 256-blocks per group
    GW = N // NG             # group width
    CHG = GW // 128          # i-chunks per group
    GZ = F_OUT + 4           # G1 row: 768 vals + Z col + pad
    ZC = F_OUT               # Z column index

    nc = bacc.Bacc("TRN2", target_bir_lowering=False, debug=False,
                   num_devices=NCORES)

    d_h = nc.dram_tensor("h", [N, F_IN], dt.float32, kind="ExternalInput")
    d_adj = nc.dram_tensor("adj", [N, N], dt.uint8, kind="ExternalInput")
    d_w = nc.dram_tensor("w", [F_IN, F_OUT], dt.float32, kind="ExternalInput")
    d_asrc = nc.dram_tensor("a_src", [F_OUT], dt.float32, kind="ExternalInput")
    d_adst = nc.dram_tensor("a_dst", [F_OUT], dt.float32, kind="ExternalInput")
    if has_bias:
        d_bias = nc.dram_tensor("bias", [F_OUT], dt.float32,
                                kind="ExternalInput")
    d_out = nc.dram_tensor("out", [N, F_OUT], dt.float32,
                           kind="ExternalOutput")

    def sig_off(tau):
        return 256 * (tau // 2) + (tau % 2)

    # Mask (g, tau) pairs: Pool takes the first KPOOLG groups entirely
    # (emitted at their gates, running under phase 2); DVE takes the rest
    # after the phase-2 loop so the dot chain isn't stalled.  The last
    # Pool group is emitted post-loop too, overlapping PV of the earlier
    # groups.
    def gate(g, tau):
        return max(4 * g + 3, tau)

    with tile.TileContext(nc) as tc:
        with tc.tile_pool(name="const", bufs=1) as cpool, \
             tc.tile_pool(name="persist", bufs=1) as pp:
            identb = cpool.tile([128, 128], dt.bfloat16, tag="identb")
            make_identity(nc, identb[:])
            ident32 = cpool.tile([128, 128], dt.float32, tag="ident32")
            make_identity(nc, ident32[:])
            eye_u8 = cpool.tile([128, 128], dt.uint8, tag="eye_u8")
            nc.vector.tensor_copy(eye_u8[:], identb[:])

            if has_bias:
                bias_row = cpool.tile([1, F_OUT], dt.float32, tag="bias_row")
                nc.sync.dma_start(bias_row[:],
                                  d_bias.ap().rearrange("(o f) -> o f", o=1))
                bias_bc = pp.tile([128, F_OUT], dt.float32, tag="bias_bc")
                nc.gpsimd.partition_broadcast(bias_bc[:], bias_row[:])

            # hT split per 4-tile n-group so phase 2 can start early
            hT = [[pp.tile([128, 512], dt.bfloat16, tag=f"hT{g2}_{k}",
                           name=f"hT{g2}_{k}")
                   for k in range(KT)] for g2 in range(NT // 4)]
            w_bf = pp.tile([128, KT, F_OUT], dt.bfloat16, tag="w_bf")
            asrc_bc = pp.tile([128, F_OUT], dt.bfloat16, tag="asrc_bc")
            adst_bc = pp.tile([128, F_OUT], dt.bfloat16, tag="adst_bc")

            stag = [pp.tile([128, N], dt.uint16, tag=f"stag{t}",
                            name=f"stag{t}")
                    for t in range(NT // 2)]
            adj16 = d_adj.ap().bitcast(dt.uint16)

            g1hi = [pp.tile([128, 2, GZ], dt.float8e4, tag=f"g1hi{s}",
                            name=f"g1hi{s}")
                    for s in range(NT // 2)]
            if KRES:
                g1res = [pp.tile([128, 2, GZ], dt.float8e4, tag=f"g1res{s}",
                                 name=f"g1res{s}")
                         for s in range(KRES8)]
            # Pt tiles: all NG groups live at once (mask building overlaps
            # phase 2 and PV consumes group-by-group)
            P2 = [[pp.tile([128, 2, GW], dt.float8e4, tag=f"P2_{g}_{s}",
                           name=f"P2_{g}_{s}")
                   for s in range(NT // 2)] for g in range(NG)]

            src_col = pp.tile([128, NT], dt.float32, tag="src_col")
            dst_col = pp.tile([128, NT], dt.float32, tag="dst_col")
            edst_col = pp.tile([128, NT], dt.float32, tag="edst_col")
            q_bc = [pp.tile([128, GW], dt.bfloat16, tag=f"q_bc{g}",
                            name=f"q_bc{g}")
                    for g in range(NG)]
            qd_col = [pp.tile([128, 4], dt.float32, tag=f"qd_col{q}",
                              name=f"qd_col{q}")
                      for q in range(NT // 4)]
            q_flat = [pp.tile([1, GW], dt.bfloat16, tag=f"q_flat{g}",
                              name=f"q_flat{g}")
                      for g in range(NG)]
            rz_col = pp.tile([128, NT], dt.float32, tag="rz_col")

            for s in range(NT // 2):
                nc.gpsimd.memset(g1hi[s][:, :, ZC + 1:], 0.0)
                if KRES and s < KRES8:
                    nc.gpsimd.memset(g1res[s][:, :, ZC + 1:], 0.0)

            def u8view(t):
                return stag[t][:].bitcast(dt.uint8).rearrange(
                    "p (b k hh o) -> p b hh k o", b=NT // 2, k=128, hh=2, o=2)

            def emit_diag(tau, dpool):
                # diag of tile tau lives in group g = tau // CHG at m-offset
                # 128*tau; recompute the 128-wide D1 diag block (cheap) and
                # write it where eye is set (the adj|eye diagonal fix)
                g = tau // CHG
                t, o = tau // 2, tau % 2
                dd = dpool.tile([128, 128], dt.float8e4, tag="dd")
                qslice = q_bc[g][:, 128 * tau - GW * g:
                                128 * (tau + 1) - GW * g]
                nc.vector.tensor_scalar(
                    dd[:], qslice,
                    qd_col[tau // 4][:, tau % 4:tau % 4 + 1], 1.0,
                    op0=OP.mult, op1=OP.max)
                nc.vector.copy_predicated(
                    P2[g][t][:, o, 128 * tau - GW * g:
                             128 * (tau + 1) - GW * g], eye_u8[:], dd[:])

            def emit_d1(g, tau, d1pool):
                d1 = d1pool.tile([128, GW], dt.bfloat16, tag="d1")
                nc.vector.tensor_scalar(
                    d1[:], q_bc[g][:],
                    qd_col[tau // 4][:, tau % 4:tau % 4 + 1], 1.0,
                    op0=OP.mult, op1=OP.max)
                return d1

            def emit_mask(g, tau, d1, on_pool):
                t, o = tau // 2, tau % 2
                u8v = u8view(t)
                d13 = d1[:].rearrange("p (b hh k) -> p b hh k",
                                      b=BB, hh=2, k=128)
                p23 = P2[g][t][:, o, :].rearrange(
                    "p (b hh k) -> p b hh k", b=BB, hh=2, k=128)
                for hh in range(2):
                    in0 = u8v[:, BB * g:BB * (g + 1), hh, :, o]
                    eng = nc.gpsimd if on_pool else nc.vector
                    eng.tensor_tensor(p23[:, :, hh, :], in0, d13[:, :, hh, :],
                                      op=OP.mult)

            with tc.tile_pool(name="ph1", bufs=1) as hpool, \
                 tc.tile_pool(name="ph1ps", bufs=2, space="PSUM") as psum1, \
                 tc.tile_pool(name="ph2", bufs=3) as tpool, \
                 tc.tile_pool(name="ph2s", bufs=3) as spool, \
                 tc.tile_pool(name="ph2ps", bufs=3, space="PSUM") as psum2:
                # ---- loads (h/w/adj on HWDGE; cast-DMAs occupy Pool, so
                # only the tiny a-vectors use them) ----
                arow = hpool.tile([1, 2 * F_OUT], dt.bfloat16, tag="arow")
                nc.gpsimd.dma_start(
                    arow[:, 0:F_OUT],
                    d_asrc.ap().rearrange("(o f) -> o f", o=1))
                nc.gpsimd.dma_start(
                    arow[:, F_OUT:],
                    d_adst.ap().rearrange("(o f) -> o f", o=1))
                nc.gpsimd.partition_broadcast(asrc_bc[:], arow[:, 0:F_OUT])
                nc.gpsimd.partition_broadcast(adst_bc[:], arow[:, F_OUT:])

                # h in sigma row order: rows 2p,2p+1 are HBM-adjacent so
                # (o f) merges into one contiguous 1536-elem run -> one DMA
                # per 4-tile group.  Order: h group 0 first (transposes),
                # then w (whole, one DMA + one DVE copy at the head of the
                # DVE queue), then the rest interleaved with adj xbar.
                dh3 = d_h.ap().rearrange("(b p o) f -> p b (o f)",
                                         b=NT // 2, p=128, o=2)
                hstage = []

                def load_h(g2):
                    hs = hpool.tile([128, 4, F_IN], dt.float32,
                                    tag=f"hs{g2 % 2}", name=f"hs{g2}")
                    nc.sync.dma_start(
                        hs[:].rearrange("p (b o) f -> p b (o f)", b=2, o=2),
                        dh3[:, 2 * g2:2 * g2 + 2, :])
                    hstage.append(hs)

                load_h(0)
                wtmp = hpool.tile([128, KT * F_OUT], dt.float32, tag="wtmp")
                nc.sync.dma_start(
                    wtmp[:].rearrange("p (k f) -> p k f", k=KT),
                    d_w.ap().rearrange("(k p) f -> p k f", k=KT, p=128))
                wt3 = wtmp[:].rearrange("p (k f) -> p k f", k=KT)
                nc.scalar.copy(w_bf[:, :, 0:512], wt3[:, :, 0:512])
                nc.scalar.copy(w_bf[:, :, 512:F_OUT], wt3[:, :, 512:F_OUT])
                for g2 in range(1, NT // 4):
                    load_h(g2)
                    for t in range(2 * (g2 - 1), 2 * (g2 - 1) + 2):
                        nc.sync.dma_start(stag[t][:],
                                          adj16[:, 128 * t:128 * (t + 1)],
                                          transpose=True)
                for t in range(NT // 2 - 2, NT // 2):
                    nc.sync.dma_start(stag[t][:],
                                      adj16[:, 128 * t:128 * (t + 1)],
                                      transpose=True)

                # ---- phase 2 tau loop with gated mask interleaving ----
                # (hT transposes/evicts are emitted just-in-time per 4-tile
                # group so the ACT queue isn't blocked on late h loads)
                emitted = set()
                diag_todo = []

                for tau in range(NT):
                    s, o = tau // 2, tau % 2
                    if tau % 4 == 0:
                        g2 = tau // 4
                        hsr = hstage[g2][:]
                        for k in range(KT):
                            ps = psum1.tile([128, 512], dt.float32,
                                            tag="tps")
                            for nn in range(4):
                                nc.tensor.transpose(
                                    ps[:, 128 * nn:128 * (nn + 1)],
                                    hsr[:, nn, 128 * k:128 * (k + 1)],
                                    ident32[:])
                            nc.scalar.copy(hT[g2][k][:], ps[:])
                    ps = psum2.tile([128, F_OUT], dt.float32, tag="hpps")
                    for k in range(KT):
                        lhsT = hT[tau // 4][k][:, 128 * (tau % 4):
                                               128 * (tau % 4 + 1)]
                        nc.tensor.matmul(ps[:, 0:512], lhsT,
                                         w_bf[:, k, 0:512],
                                         start=(k == 0), stop=(k == KT - 1))
                        nc.tensor.matmul(ps[:, 512:F_OUT], lhsT,
                                         w_bf[:, k, 512:F_OUT],
                                         start=(k == 0), stop=(k == KT - 1))
                    tnh = tpool.tile([128, F_OUT], dt.bfloat16, tag="tnh")
                    nc.scalar.activation(tnh[:], ps[:], AF.Tanh)
                    scr = tpool.tile([128, F_OUT], dt.bfloat16, tag="scr")
                    nc.vector.scalar_tensor_tensor(
                        scr[:], tnh[:], 1.0, asrc_bc[:],
                        op0=OP.mult, op1=OP.mult,
                        accum_out=src_col[:, tau:tau + 1])
                    scr2 = tpool.tile([128, F_OUT], dt.bfloat16, tag="scr2")
                    nc.vector.scalar_tensor_tensor(
                        scr2[:], tnh[:], 1.0, adst_bc[:],
                        op0=OP.mult, op1=OP.mult,
                        accum_out=dst_col[:, tau:tau + 1])
                    nc.scalar.activation(edst_col[:, tau:tau + 1],
                                         dst_col[:, tau:tau + 1], AF.Exp)
                    nc.scalar.activation(
                        qd_col[tau // 4][:, tau % 4:tau % 4 + 1],
                        dst_col[:, tau:tau + 1], AF.Exp, scale=-0.8)
                    if has_bias:
                        nc.gpsimd.tensor_tensor(ps[:], ps[:], bias_bc[:],
                                                op=OP.add)
                    esl = edst_col[:, tau:tau + 1]
                    g1bf = tpool.tile([128, ZC + 1], dt.bfloat16, tag="g1bf")
                    nc.scalar.activation(g1bf[:, 0:F_OUT], ps[:],
                                         AF.Copy, scale=esl)
                    nc.scalar.copy(g1bf[:, ZC:ZC + 1], esl)
                    # f8 quantize on DVE (2x), residual on Pool
                    nc.vector.tensor_copy(g1hi[s][:, o, 0:ZC + 1], g1bf[:])
                    if KRES and s < KRES8:
                        nc.gpsimd.tensor_tensor(
                            g1res[s][:, o, 0:ZC + 1], g1bf[:],
                            g1hi[s][:, o, 0:ZC + 1], op=OP.subtract)
                    # q row slice for group g=tau//4 once its src dots exist
                    if tau % 4 == 3:
                        g = tau // 4
                        qb = spool.tile([128, 4], dt.bfloat16, tag="qb")
                        nc.scalar.activation(qb[:], src_col[:, 4 * g:4 * g + 4],
                                             AF.Exp, scale=-0.8)
                        psqt = psum1.tile([128, 512], dt.float32,
                                          tag="tps", name=f"psq{g}")
                        psq = psqt[0:4, 0:64].bitcast(dt.bfloat16)
                        nc.tensor.transpose(psq, qb[:], identb[:])
                        qsT = spool.tile([4, 128], dt.bfloat16, tag="qsT")
                        nc.scalar.copy(qsT[:], psq)
                        nc.sync.dma_start(
                            q_flat[g][:].rearrange("o (t p) -> o t p", t=4),
                            qsT[:])
                        nc.gpsimd.partition_broadcast(q_bc[g][:],
                                                      q_flat[g][:])

                # Post-loop: masks per group (alternating DVE/Pool),
                # staggered with PV emission so early PV chains don't sit
                # behind the last group's q-broadcast in the coarse
                # engine-counter waits.
                KPOOLM = int(os.environ.get("KPOOLM", "5"))

                def emit_group_masks(g, mpool):
                    for tp in range(NT):
                        on_pool = (tp % 16) < KPOOLM
                        emit_mask(g, tp, emit_d1(g, tp, mpool), on_pool)
                        emitted.add((g, tp))
                        if tp // CHG == g:
                            emit_diag(tp, mpool)

            # ---- PV: DoubleRow chains per (g, i-chunk) ----
            with tc.tile_pool(name="pgo", bufs=4) as opool, \
                 tc.tile_pool(name="pgm", bufs=3) as mpool, \
                 tc.tile_pool(name="pgps", bufs=4, space="PSUM") as psum5:
                FH = ((512, GZ - 2), (0, 512))  # Z-half first

                def emit_pv(g):
                    for c in range(CHG):
                        ci = CHG * g + c
                        pso = psum5.tile([128, GZ], dt.float32, tag="pso")
                        nchain = NT // 2 + (KRES8 if KRES else 0)
                        for fh, (f0, f1) in enumerate(FH):
                            idx = 0
                            for s in range(NT // 2):
                                nc.tensor.matmul(
                                    pso[:, f0:f1],
                                    P2[g][s][:, :, 128 * c:128 * (c + 1)],
                                    g1hi[s][:, :, f0:f1],
                                    start=(idx == 0),
                                    stop=(idx == nchain - 1),
                                    perf_mode=PM.DoubleRow)
                                idx += 1
                            if KRES:
                                for s in range(KRES8):
                                    nc.tensor.matmul(
                                        pso[:, f0:f1],
                                        P2[g][s][:, :, 128 * c:128 * (c + 1)],
                                        g1res[s][:, :, f0:f1],
                                        start=False,
                                        stop=(idx == nchain - 1),
                                        perf_mode=PM.DoubleRow)
                                    idx += 1
                            if fh == 0:
                                nc.vector.reciprocal(
                                    rz_col[:, ci:ci + 1], pso[:, ZC:ZC + 1])
                        ob = opool.tile([128, F_OUT], dt.float32, tag="ob")
                        nc.scalar.activation(
                            ob[:], pso[:, 0:F_OUT], AF.Copy,
                            scale=rz_col[:, ci:ci + 1])
                        base = sig_off(ci)
                        nc.sync.dma_start(d_out[base:base + 255:2, :], ob[:])

                KDRIVER = os.environ.get("KDRIVER", "MMPMPMPP")
                mq, pq = [0, 1, 2, 3], [0, 1, 2, 3]
                for ch in KDRIVER:
                    if ch == 'M' and mq:
                        emit_group_masks(mq.pop(0), mpool)
                    elif ch == 'P' and pq:
                        emit_pv(pq.pop(0))
                for g in mq:
                    emit_group_masks(g, mpool)
                for g in pq:
                    emit_pv(g)

    nc.compile()
    return nc


def _get_program(has_bias: bool):
    key = ("prog", has_bias)
    if key not in _CACHE:
        _CACHE[key] = _build(has_bias)
    return _CACHE[key]


def kernel(h, adj, w, a_src, a_dst, bias):
    from concourse.bass_utils import run_bass_kernel_spmd

    h = np.ascontiguousarray(np.asarray(h, dtype=np.float32))
    adj_u8 = np.ascontiguousarray(np.asarray(adj).astype(np.uint8))
    w = np.ascontiguousarray(np.asarray(w, dtype=np.float32))
    a_src = np.asarray(a_src, dtype=np.float32).reshape(H, F_OUT)
    a_dst = np.asarray(a_dst, dtype=np.float32).reshape(H, F_OUT)
    bias = np.asarray(bias, dtype=np.float32).reshape(F_OUT)
    has_bias = bool(np.any(bias))

    nc = _get_program(has_bias)

    in_maps = []
    for core in range(NCORES):
        b, hd = core // H, core % H
        m = {
            "h": h[b],
            "adj": adj_u8[b],
            "w": w[hd],
            "a_src": a_src[hd],
            "a_dst": a_dst[hd],
        }
        if has_bias:
            m["bias"] = bias
        in_maps.append(m)

    res = run_bass_kernel_spmd(nc, in_maps, list(range(NCORES)))
    out = np.empty((BS, H, N, F_OUT), dtype=np.float32)
    for core in range(NCORES):
        b, hd = core // H, core % H
        out[b, hd] = res.results[core]["out"]
    return out
